# revision 24
# baseline (speedup 1.0000x reference)
"""Trainium2 Bass kernel for GraphTransformer sparse attention (v4).

Strategy (8 NeuronCores, SPMD):
  - dst nodes grouped into 128-dst chunks. Chunks are sorted by edge count and
    dealt to the 8 cores rank-block-wise, so all cores share one program with a
    per-position group count J_i (less pad than a global max).
  - Host packs per-slot sequential bf16 streams (slot grid rows, pad ld=-1):
      evq [T, 768] = [e | v[src] | qn[dst]] ; knp [T, 256] = kn[src] ;
      ldp f32 [T]. Big HWDGE DMAs; no indirect gather.
  - Device per chunk: ACT copies e into ke tile, SWDGE accumulate-DMA adds kn
    (split at the 8-group boundary: accumulate corrupts dest runs >4096B per
    partition). ve=v+e, prod=qx*ke, s=2-stage reduce, pv=p*ve with
    ACT-broadcast p (all DVE; GpSimd does only DMA emission since its
    elementwise contends with DVE SBUF ports). One-hot matmuls accumulate
    rhs=[pv|p] into PSUM [acc|l]; device ships raw [acc|l] (f32); host does
    the l-normalization (and the empty-dst guard).
"""
import numpy as np
from contextlib import ExitStack

import ml_dtypes

import concourse.bass as bass
import concourse.bacc as bacc
import concourse.mybir as mybir
import concourse.tile as tile
from concourse.bass_utils import run_bass_kernel_spmd

N, E, H, C = 50000, 400000, 8, 32
HC = H * C                      # 256
NCORES = 8
D = 128                         # dsts per chunk
EPS = 1e-6
QK_SCALE = 1.0 / np.sqrt(np.float32(C))

F32 = mybir.dt.float32
F16 = mybir.dt.float16
BF16 = mybir.dt.bfloat16
BF = ml_dtypes.bfloat16

_cache = {}
_last_launch = None


def _build_program(J_list):
    """SPMD Bass program. J_list[i] = 128-slot groups for chunk position i."""
    Jmax = max(J_list)
    T = sum(J_list) * 128       # total slot rows per core
    cpc = len(J_list)
    nc = bacc.Bacc()

    evq = nc.declare_dram_parameter("evq", [T, 3 * HC], BF16, isOutput=False)
    knp = nc.declare_dram_parameter("knp", [T, HC], BF16, isOutput=False)
    ldp = nc.declare_dram_parameter("ldp", [T], F32, isOutput=False)
    iota_row = nc.declare_dram_parameter("iota_row", [128, 128], F32, isOutput=False)
    out = nc.declare_dram_parameter("out", [cpc * D, HC + H], BF16, isOutput=True)

    with tile.TileContext(nc) as tc, ExitStack() as ctx:
        consts = ctx.enter_context(tc.tile_pool(name="consts", bufs=1))
        big = ctx.enter_context(tc.tile_pool(name="big", bufs=4))
        med = ctx.enter_context(tc.tile_pool(name="med", bufs=3))
        small = ctx.enter_context(tc.tile_pool(name="small", bufs=3))
        pp_acc = ctx.enter_context(tc.tile_pool(name="pp_acc", bufs=3, space="PSUM"))

        iota_t = consts.tile([128, 128], F32)
        nc.sync.dma_start(iota_t[:], iota_row[:])

        off = 0
        for i, J in enumerate(J_list):
            r0, r1 = off * 128, (off + J) * 128
            off += J

            # ---- loads: [e|v|qx] mega-stream; ke = copy(e); ke += kn ----
            in_t = big.tile([128, Jmax, 3 * HC], BF16, tag="in")
            nc.sync.dma_start(
                in_t[:, 0:J, :], evq[r0:r1].rearrange("(p j) d -> p j d", p=128))
            e_s = in_t[:, 0:J, 0:HC]
            v_s = in_t[:, 0:J, HC:2 * HC]
            qx_s = in_t[:, 0:J, 2 * HC:3 * HC]
            ld_t = small.tile([128, Jmax], F32, tag="ld")
            nc.sync.dma_start(
                ld_t[:, 0:J], ldp[r0:r1].rearrange("(p j) -> p j", p=128))

            ke_t = med.tile([128, Jmax, HC], BF16, tag="ke")
            nc.scalar.copy(ke_t[:, 0:J, :], e_s)
            kn_src = knp[r0:r1].rearrange("(p j) d -> p j d", p=128)
            for a, b in ((0, min(J, 8)), (8, J)):
                if b <= a:
                    continue
                nc.gpsimd.dma_start(
                    ke_t[:, a:b, :], kn_src[:, a:b, :],
                    accum_op=mybir.AluOpType.add)

            # ---- ve = v + e ----
            ve_t = med.tile([128, Jmax, HC], BF16, tag="ve")
            nc.vector.tensor_add(ve_t[:, 0:J, :], v_s, e_s)

            # ---- scores: prod = qx*ke ; s = 2-stage reduce ; p = exp(s) ----
            prod_t = med.tile([128, Jmax, HC], BF16, tag="prod")
            nc.vector.tensor_mul(prod_t[:, 0:J, :], qx_s, ke_t[:, 0:J, :])
            p4 = prod_t.rearrange("p j (h c) -> p j h c", c=C)
            t1_t = med.tile([128, Jmax, H, 16], F16, tag="t1")
            nc.vector.tensor_add(
                t1_t[:, 0:J], p4[:, 0:J, :, 0:16], p4[:, 0:J, :, 16:32])
            s_t = small.tile([128, Jmax, H], F32, tag="s")
            nc.vector.tensor_reduce(
                out=s_t[:, 0:J, :],
                in_=t1_t.rearrange("p j h c -> p (j h) c")[:, 0:J * H, :],
                axis=mybir.AxisListType.X, op=mybir.AluOpType.add)

            # ---- pvp = [pv | p], pv = ve * ACT-expanded p ----
            pvp_t = big.tile([128, Jmax, HC + H], BF16, tag="pvp")
            nc.scalar.activation(
                pvp_t[:, 0:J, HC:HC + H], s_t[:, 0:J, :],
                mybir.ActivationFunctionType.Exp)
            p_col = pvp_t[:, :, HC:HC + H]
            pexp_t = med.tile([128, Jmax, H, C], BF16, tag="pexp")
            nc.scalar.copy(
                pexp_t[:, 0:J],
                p_col[:, 0:J, :, None].to_broadcast([128, J, H, C]))
            nc.vector.tensor_mul(
                pvp_t[:, 0:J, 0:HC].rearrange("p j (h c) -> p j h c", c=C),
                ve_t.rearrange("p j (h c) -> p j h c", c=C)[:, 0:J],
                pexp_t[:, 0:J])

            # ---- one-hot (single TT) ----
            oh_t = med.tile([128, Jmax, 128], BF16, tag="oh")
            nc.vector.tensor_tensor(
                oh_t[:, 0:J, :],
                ld_t[:, 0:J, None].to_broadcast([128, J, 128]),
                iota_t[:, None, :].to_broadcast([128, J, 128]),
                mybir.AluOpType.is_equal)

            # ---- segment sums into PSUM: [acc | l] ----
            acc_ps = pp_acc.tile([128, HC + H], F32, tag="acc")
            for j in range(J):
                nc.tensor.matmul(
                    acc_ps[:], lhsT=oh_t[:, j, :], rhs=pvp_t[:, j, :],
                    start=(j == 0), stop=(j == J - 1))

            # ---- ship raw [acc | l]; host divides ----
            ao_t = small.tile([128, HC + H], BF16, tag="ao")
            nc.scalar.copy(ao_t[:], acc_ps[:])
            nc.sync.dma_start(out[i * D:(i + 1) * D, :], ao_t[:])

    nc.compile()
    return nc


def kernel(q, k, v, e, w_q_norm, w_k_norm, edge_src, edge_dst):
    q = np.asarray(q, np.float32).reshape(N, HC)
    k = np.asarray(k, np.float32).reshape(N, HC)
    v = np.asarray(v, np.float32).reshape(N, HC)
    e = np.asarray(e, np.float32).reshape(E, HC)
    wq = np.asarray(w_q_norm, np.float32)
    wk = np.asarray(w_k_norm, np.float32)
    edge_src = np.asarray(edge_src, np.int64)
    edge_dst = np.asarray(edge_dst, np.int64)

    # host: rms-norm node tables (O(N) math; per-edge work is indexing only)
    def rms(x, w):
        x3 = x.reshape(-1, H, C)
        r = x3 / np.sqrt((x3 * x3).mean(-1, keepdims=True) + EPS)
        return (r * w[None, None, :]).reshape(-1, HC).astype(np.float32)

    kn16 = rms(k, wk).astype(BF)
    qn16 = (rms(q, wq) * np.float32(QK_SCALE)).astype(BF)
    v16 = v.astype(BF)
    e16 = e.astype(BF)

    # chunking: sort chunks by edge count, deal rank-blocks to cores
    n_chunks = (N + D - 1) // D                      # 391
    cpc = (n_chunks + NCORES - 1) // NCORES          # 49
    nch = cpc * NCORES                               # 392
    starts = np.searchsorted(edge_dst, np.arange(0, (nch + 1) * D, D)).astype(np.int64)
    counts = np.diff(starts)
    order = np.argsort(-counts, kind="stable")
    J_list = tuple(
        max(1, int(np.ceil(counts[order[i * NCORES]] / 128))) for i in range(cpc))
    offs = np.concatenate([[0], np.cumsum(np.array(J_list) * 128)]).astype(np.int64)
    T = int(offs[-1])

    key = J_list
    if key not in _cache:
        _cache[key] = _build_program(list(J_list))
    nc = _cache[key]

    # per-edge slot address
    pos_of_chunk = np.empty(nch, np.int64)
    core_of_chunk = np.empty(nch, np.int64)
    pos_of_chunk[order] = np.arange(nch) // NCORES
    core_of_chunk[order] = np.arange(nch) % NCORES
    c_of_e = edge_dst >> 7
    epos = np.arange(E, dtype=np.int64) - starts[c_of_e]
    erow = offs[pos_of_chunk[c_of_e]] + epos
    ecore = core_of_chunk[c_of_e]
    eslot = ecore * T + erow

    evq = np.zeros((NCORES * T, 3 * HC), BF)
    evq[eslot, 0:HC] = e16
    evq[eslot, HC:2 * HC] = v16[edge_src]
    evq[eslot, 2 * HC:3 * HC] = qn16[edge_dst]
    knp = np.zeros((NCORES * T, HC), BF)
    knp[eslot] = kn16[edge_src]
    ldp = np.full((NCORES * T,), -1.0, np.float32)
    ldp[eslot] = (edge_dst - (c_of_e << 7)).astype(np.float32)
    iota_row = np.tile(np.arange(128, dtype=np.float32)[None, :], (128, 1))

    in_maps = []
    for m in range(NCORES):
        in_maps.append({
            "evq": evq[m * T:(m + 1) * T], "knp": knp[m * T:(m + 1) * T],
            "ldp": ldp[m * T:(m + 1) * T], "iota_row": iota_row,
        })

    global _last_launch
    _last_launch = (nc, in_maps)
    res = run_bass_kernel_spmd(nc, in_maps, list(range(NCORES)))
    outs = [np.asarray(res.results[m]["out"]) for m in range(NCORES)]
    full = np.zeros((N, HC), np.float32)
    for g in range(n_chunks):
        i, m = pos_of_chunk[g], core_of_chunk[g]
        lo = g * D
        hi = min(lo + D, N)
        raw = outs[m][i * D:i * D + (hi - lo)].astype(np.float32)
        acc = raw[:, 0:HC].reshape(-1, H, C)
        l = raw[:, HC:HC + H]
        o = np.where(l[:, :, None] > 0, acc / np.maximum(l, 1e-30)[:, :, None], 0.0)
        full[lo:hi] = o.reshape(-1, HC)
    return full.reshape(N, H, C)


# revision 26
# speedup vs baseline: 1.1017x; 1.1017x over previous
"""Trainium2 Bass kernel for GraphTransformer sparse attention (v4).

Strategy (8 NeuronCores, SPMD):
  - dst nodes grouped into 128-dst chunks. Chunks are sorted by edge count and
    dealt to the 8 cores rank-block-wise, so all cores share one program with a
    per-position group count J_i (less pad than a global max).
  - Host packs per-slot sequential bf16 streams (slot grid rows, pad ld=-1):
      evq [T, 768] = [e | v[src] | qn[dst]] ; knp [T, 256] = kn[src] ;
      ldp f32 [T]. Big HWDGE DMAs; no indirect gather.
  - Device per chunk: ACT copies e into ke tile, SWDGE accumulate-DMA adds kn
    (split at the 8-group boundary: accumulate corrupts dest runs >4096B per
    partition). ve=v+e, prod=qx*ke, s=2-stage reduce, pv=p*ve with
    ACT-broadcast p (all DVE; GpSimd does only DMA emission since its
    elementwise contends with DVE SBUF ports). One-hot matmuls accumulate
    rhs=[pv|p] into PSUM [acc|l]; device ships raw [acc|l] (f32); host does
    the l-normalization (and the empty-dst guard).
"""
import numpy as np
from contextlib import ExitStack

import ml_dtypes

import concourse.bass as bass
import concourse.bacc as bacc
import concourse.mybir as mybir
import concourse.tile as tile
from concourse.bass_utils import run_bass_kernel_spmd

N, E, H, C = 50000, 400000, 8, 32
HC = H * C                      # 256
NCORES = 8
D = 128                         # dsts per chunk
EPS = 1e-6
QK_SCALE = 1.0 / np.sqrt(np.float32(C))

F32 = mybir.dt.float32
F16 = mybir.dt.float16
BF16 = mybir.dt.bfloat16
BF = ml_dtypes.bfloat16

_cache = {}
_last_launch = None


def _build_program(J_list):
    """SPMD Bass program. J_list[i] = 128-slot groups for chunk position i."""
    Jmax = max(J_list)
    T = sum(J_list) * 128       # total slot rows per core
    cpc = len(J_list)
    nc = bacc.Bacc()

    evq = nc.declare_dram_parameter("evq", [T, 3 * HC], BF16, isOutput=False)
    knp = nc.declare_dram_parameter("knp", [T, HC], BF16, isOutput=False)
    ldp = nc.declare_dram_parameter("ldp", [T], F32, isOutput=False)
    iota_row = nc.declare_dram_parameter("iota_row", [128, 128], F32, isOutput=False)
    out = nc.declare_dram_parameter("out", [cpc * D, HC + H], BF16, isOutput=True)

    with tile.TileContext(nc) as tc, ExitStack() as ctx:
        consts = ctx.enter_context(tc.tile_pool(name="consts", bufs=1))
        big = ctx.enter_context(tc.tile_pool(name="big", bufs=4))
        med = ctx.enter_context(tc.tile_pool(name="med", bufs=3))
        small = ctx.enter_context(tc.tile_pool(name="small", bufs=2))
        pp_acc = ctx.enter_context(tc.tile_pool(name="pp_acc", bufs=4, space="PSUM"))

        iota_t = consts.tile([128, 128], F32)
        nc.sync.dma_start(iota_t[:], iota_row[:])

        off = 0
        for i, J in enumerate(J_list):
            r0, r1 = off * 128, (off + J) * 128
            off += J

            # ---- loads: [e|v|qx] mega-stream; ke = copy(e); ke += kn ----
            in_t = big.tile([128, Jmax, 3 * HC], BF16, tag="in")
            nc.sync.dma_start(
                in_t[:, 0:J, :], evq[r0:r1].rearrange("(p j) d -> p j d", p=128))
            e_s = in_t[:, 0:J, 0:HC]
            v_s = in_t[:, 0:J, HC:2 * HC]
            qx_s = in_t[:, 0:J, 2 * HC:3 * HC]
            ld_t = small.tile([128, Jmax], F32, tag="ld")
            nc.sync.dma_start(
                ld_t[:, 0:J], ldp[r0:r1].rearrange("(p j) -> p j", p=128))

            ke_t = med.tile([128, Jmax, HC], BF16, tag="ke")
            nc.scalar.copy(ke_t[:, 0:J, :], e_s)
            kn_src = knp[r0:r1].rearrange("(p j) d -> p j d", p=128)
            for a, b in ((0, min(J, 8)), (8, J)):
                if b <= a:
                    continue
                nc.gpsimd.dma_start(
                    ke_t[:, a:b, :], kn_src[:, a:b, :],
                    accum_op=mybir.AluOpType.add)

            # ---- ve = v + e ----
            ve_t = med.tile([128, Jmax, HC], BF16, tag="ve")
            nc.vector.tensor_add(ve_t[:, 0:J, :], v_s, e_s)

            # ---- scores: prod = qx*ke ; s = 2-stage reduce ; p = exp(s) ----
            prod_t = med.tile([128, Jmax, HC], BF16, tag="prod")
            nc.vector.tensor_mul(prod_t[:, 0:J, :], qx_s, ke_t[:, 0:J, :])
            p4 = prod_t.rearrange("p j (h c) -> p j h c", c=C)
            t1_t = med.tile([128, Jmax, H, 16], F16, tag="t1")
            nc.vector.tensor_add(
                t1_t[:, 0:J], p4[:, 0:J, :, 0:16], p4[:, 0:J, :, 16:32])
            s_t = small.tile([128, Jmax, H], F32, tag="s")
            nc.vector.tensor_reduce(
                out=s_t[:, 0:J, :],
                in_=t1_t.rearrange("p j h c -> p (j h) c")[:, 0:J * H, :],
                axis=mybir.AxisListType.X, op=mybir.AluOpType.add)

            # ---- pvp = [pv | p], pv = ve * ACT-expanded p ----
            pvp_t = big.tile([128, Jmax, HC + H], BF16, tag="pvp")
            nc.scalar.activation(
                pvp_t[:, 0:J, HC:HC + H], s_t[:, 0:J, :],
                mybir.ActivationFunctionType.Exp)
            p_col = pvp_t[:, :, HC:HC + H]
            pexp_t = med.tile([128, Jmax, H, C], BF16, tag="pexp")
            nc.scalar.copy(
                pexp_t[:, 0:J],
                p_col[:, 0:J, :, None].to_broadcast([128, J, H, C]))
            nc.vector.tensor_mul(
                pvp_t[:, 0:J, 0:HC].rearrange("p j (h c) -> p j h c", c=C),
                ve_t.rearrange("p j (h c) -> p j h c", c=C)[:, 0:J],
                pexp_t[:, 0:J])

            # ---- one-hot (single TT) ----
            oh_t = med.tile([128, Jmax, 128], BF16, tag="oh")
            nc.vector.tensor_tensor(
                oh_t[:, 0:J, :],
                ld_t[:, 0:J, None].to_broadcast([128, J, 128]),
                iota_t[:, None, :].to_broadcast([128, J, 128]),
                mybir.AluOpType.is_equal)

            # ---- segment sums into PSUM: [acc | l] ----
            acc_ps = pp_acc.tile([128, HC + H], F32, tag="acc")
            for j in range(J):
                nc.tensor.matmul(
                    acc_ps[:], lhsT=oh_t[:, j, :], rhs=pvp_t[:, j, :],
                    start=(j == 0), stop=(j == J - 1))

            # ---- ship raw [acc | l]; host divides ----
            ao_t = small.tile([128, HC + H], BF16, tag="ao")
            nc.scalar.copy(ao_t[:], acc_ps[:])
            nc.sync.dma_start(out[i * D:(i + 1) * D, :], ao_t[:])

    nc.compile()
    return nc


def kernel(q, k, v, e, w_q_norm, w_k_norm, edge_src, edge_dst):
    q = np.asarray(q, np.float32).reshape(N, HC)
    k = np.asarray(k, np.float32).reshape(N, HC)
    v = np.asarray(v, np.float32).reshape(N, HC)
    e = np.asarray(e, np.float32).reshape(E, HC)
    wq = np.asarray(w_q_norm, np.float32)
    wk = np.asarray(w_k_norm, np.float32)
    edge_src = np.asarray(edge_src, np.int64)
    edge_dst = np.asarray(edge_dst, np.int64)

    # host: rms-norm node tables (O(N) math; per-edge work is indexing only)
    def rms(x, w):
        x3 = x.reshape(-1, H, C)
        r = x3 / np.sqrt((x3 * x3).mean(-1, keepdims=True) + EPS)
        return (r * w[None, None, :]).reshape(-1, HC).astype(np.float32)

    kn16 = rms(k, wk).astype(BF)
    qn16 = (rms(q, wq) * np.float32(QK_SCALE)).astype(BF)
    v16 = v.astype(BF)
    e16 = e.astype(BF)

    # chunking: sort chunks by edge count, deal rank-blocks to cores
    n_chunks = (N + D - 1) // D                      # 391
    cpc = (n_chunks + NCORES - 1) // NCORES          # 49
    nch = cpc * NCORES                               # 392
    starts = np.searchsorted(edge_dst, np.arange(0, (nch + 1) * D, D)).astype(np.int64)
    counts = np.diff(starts)
    order = np.argsort(-counts, kind="stable")
    J_list = tuple(
        max(1, int(np.ceil(counts[order[i * NCORES]] / 128))) for i in range(cpc))
    offs = np.concatenate([[0], np.cumsum(np.array(J_list) * 128)]).astype(np.int64)
    T = int(offs[-1])

    key = J_list
    if key not in _cache:
        _cache[key] = _build_program(list(J_list))
    nc = _cache[key]

    # per-edge slot address
    pos_of_chunk = np.empty(nch, np.int64)
    core_of_chunk = np.empty(nch, np.int64)
    pos_of_chunk[order] = np.arange(nch) // NCORES
    core_of_chunk[order] = np.arange(nch) % NCORES
    c_of_e = edge_dst >> 7
    epos = np.arange(E, dtype=np.int64) - starts[c_of_e]
    erow = offs[pos_of_chunk[c_of_e]] + epos
    ecore = core_of_chunk[c_of_e]
    eslot = ecore * T + erow

    evq = np.zeros((NCORES * T, 3 * HC), BF)
    evq[eslot, 0:HC] = e16
    evq[eslot, HC:2 * HC] = v16[edge_src]
    evq[eslot, 2 * HC:3 * HC] = qn16[edge_dst]
    knp = np.zeros((NCORES * T, HC), BF)
    knp[eslot] = kn16[edge_src]
    ldp = np.full((NCORES * T,), -1.0, np.float32)
    ldp[eslot] = (edge_dst - (c_of_e << 7)).astype(np.float32)
    iota_row = np.tile(np.arange(128, dtype=np.float32)[None, :], (128, 1))

    in_maps = []
    for m in range(NCORES):
        in_maps.append({
            "evq": evq[m * T:(m + 1) * T], "knp": knp[m * T:(m + 1) * T],
            "ldp": ldp[m * T:(m + 1) * T], "iota_row": iota_row,
        })

    global _last_launch
    _last_launch = (nc, in_maps)
    res = run_bass_kernel_spmd(nc, in_maps, list(range(NCORES)))
    outs = [np.asarray(res.results[m]["out"]) for m in range(NCORES)]
    full = np.zeros((N, HC), np.float32)
    for g in range(n_chunks):
        i, m = pos_of_chunk[g], core_of_chunk[g]
        lo = g * D
        hi = min(lo + D, N)
        raw = outs[m][i * D:i * D + (hi - lo)].astype(np.float32)
        acc = raw[:, 0:HC].reshape(-1, H, C)
        l = raw[:, HC:HC + H]
        o = np.where(l[:, :, None] > 0, acc / np.maximum(l, 1e-30)[:, :, None], 0.0)
        full[lo:hi] = o.reshape(-1, HC)
    return full.reshape(N, H, C)
